# revision 1
# baseline (speedup 1.0000x reference)
"""AttentionBlock (GroupNorm + single-head self-attention + residual) as a
Bass/Tile kernel for one Trainium2 chip (8 NeuronCores), SPMD data-parallel.

fp8/bf16 redesign of the f32r baseline:
- QKV projections run in bf16 (x and weights bf16; fp8 here would push the
  worst-case rel err near the 2e-2 gate on wide-score datasets). Their
  outputs are cast to fp8e4 at PSUM evacuation.
- Attention matmuls (scores, PV, out-proj) run in float8e4 (e4m3) with
  MatmulPerfMode.DoubleRow: the 256-deep contraction is packed as two
  128-row matrices per call at 0.5 PE cycles/row.
- Softmax over the partition axis (S^T layout): exp on ACT in [128,1024]
  batches straight out of paired PSUM banks; exp arg is SCALE*s - 3.5 so
  fp8 pt can't overflow (ACT's fp8 cast yields inf, not saturation, above
  240; shift-invariance keeps softmax exact). The denominator is an
  all-ones DoubleRow matmul accumulated on PE alongside PV — it reduces
  over j AND broadcasts the sum to all 128 partitions (replaces the
  baseline's DVE accumulation tree + ones_sq broadcast matmul).
- GroupNorm stats come from every other column of the bf16 x copy
  (sampling error ~0.3% of sigma, far below tolerance) so the stats chain
  clears ~4us after the x DMA. rsqrt = ACT Sqrt + DVE reciprocal;
  identity/copy live in every ACT table so the only table switch is
  sqrt -> exp, once.
- V's bias is never applied on-chip: sum_j attn = 1, so it folds into the
  output-projection bias (b_fp = bp + wp.T @ (bv + wv.T @ gn_bias)).
- K (g-major) and Q stream group-wise; attention chunk 0 starts once the
  first K/Q groups land, and the remaining K/Q/V work is interleaved into
  the chunk-0/1 pair slots (PE has ~2x slack under the ACT exp stream).

Sharding: 4 images x 2 query-halves -> 8 cores. x is pre-rolled per half
on the host (keys are permutation-invariant); residual/output use the
original column range h*L..(h+1)*L.
"""

import numpy as np

import bass_rust
import concourse.bass as bass
import concourse.mybir as mybir
import concourse.tile as tile
from concourse.bass import ts
from concourse.bass_utils import run_bass_kernel_spmd

# ---------------------------------------------------------------------------
# walrus single-sync-wait workaround (same as baseline)

_counter = [0]


def _mk_nop(engine, wait):
    _counter[0] += 1
    nop = mybir.InstNoOp(name=f"WSPLIT-{_counter[0]}", ins=[], outs=[])
    nop.engine = engine
    nop.sync_info = bass_rust.SyncInfo(on_wait=[wait], on_update=[])
    return nop


def split_waits(nc, verbose=False):
    f = nc.m.functions[0]
    new_blocks = []
    n_split = 0
    for blk in f.blocks:
        insts = blk.instructions
        out = []
        for inst in insts:
            si = inst.sync_info
            if si is not None and si.on_wait and len(si.on_wait) > 1:
                waits = list(si.on_wait)
                for w in waits[1:]:
                    out.append(_mk_nop(inst.engine, w))
                si.on_wait = waits[:1]
                n_split += 1
            out.append(inst)
        new_blocks.append(bass_rust.BasicBlock(name=blk.name, instructions=out))
    f.blocks = new_blocks
    if verbose:
        print(f"split_waits: split {n_split} instructions")
    return n_split


# ---------------------------------------------------------------------------

DT = mybir.dt.float32
DB = mybir.dt.bfloat16
D8 = mybir.dt.float8e4
AF = mybir.ActivationFunctionType
OP = mybir.AluOpType
DRM = mybir.MatmulPerfMode.DoubleRow

C = 256
N = 4096
L = 2048
IC = 512          # i-chunk size
NCH = L // IC     # 4 chunks
NJT = N // 128    # 32 j-tiles
NJP = NJT // 2    # 16 j-tile pairs
CT = C // 128     # 2 channel tiles
GROUPS = 8
EPS = 1e-5
SCALE = C ** -0.5
SHIFT = -4.5


def build(split=True, repeat=1, prec=None, debug=False):
    nc = bass.Bass()
    dbg_d = {}
    if debug:
        dbg_d["d_scale"] = nc.declare_dram_parameter("d_scale", [128, 2 * CT], DT, isOutput=True)
        dbg_d["d_k8"] = nc.declare_dram_parameter("d_k8", [128, CT * N], DT, isOutput=True)
        dbg_d["d_q8"] = nc.declare_dram_parameter("d_q8", [128, CT * L], DT, isOutput=True)
        dbg_d["d_vt8"] = nc.declare_dram_parameter("d_vt8", [128, 2 * 512], DT, isOutput=True)
        dbg_d["d_pt"] = nc.declare_dram_parameter("d_pt", [128, 1024], DT, isOutput=True)
        dbg_d["d_den"] = nc.declare_dram_parameter("d_den", [128, IC], DT, isOutput=True)
        dbg_d["d_ao"] = nc.declare_dram_parameter("d_ao", [128, 2 * IC], DT, isOutput=True)

    # xb: full image, bf16, ct-major free dim: [p, t*N + j] = x[t*128+p, j]
    xb_d = nc.declare_dram_parameter("xb", [128, CT * N], DB, isOutput=False)
    # x8: same data pre-cast to fp8 on the host (V projection operand)
    x8_d = nc.declare_dram_parameter("x8", [128, CT * N], D8, isOutput=False)
    # xh: fp32 residual slice (this core's query half): [t, p, i]
    xh_d = nc.declare_dram_parameter("xh", [CT, 128, L], DT, isOutput=False)
    # weights, transposed block layout: [p, t*C + o] = w[o, t*128+p]
    wq_d = nc.declare_dram_parameter("wqT", [128, CT * C], DB, isOutput=False)
    wk_d = nc.declare_dram_parameter("wkT", [128, CT * C], DB, isOutput=False)
    wv_d = nc.declare_dram_parameter("wvT", [128, CT * C], DB, isOutput=False)
    wp_d = nc.declare_dram_parameter("wpT", [128, CT * C], DB, isOutput=False)
    # packed small params: per channel-tile [bq bk bv bp gnw gnb | G(8)] = 14
    bias6_d = nc.declare_dram_parameter("bias6", [CT, 128, 14], DT, isOutput=False)
    gt_d = nc.declare_dram_parameter("GT", [GROUPS, CT * 128], DT, isOutput=False)
    y_d = nc.declare_dram_parameter("y", [CT, 128, L], DT, isOutput=True)

    with tile.TileContext(nc) as tc:
        with (
            tc.tile_pool(name="io", bufs=1) as io,
            tc.tile_pool(name="wp_", bufs=1) as wpool,
            tc.tile_pool(name="kvq", bufs=1) as kvq,
            tc.tile_pool(name="ptp", bufs=6) as ptp,
            tc.tile_pool(name="mis", bufs=4) as mis,
            tc.tile_pool(name="ps_big", bufs=2, space="PSUM") as ps_big,
            tc.tile_pool(name="ps_pv", bufs=1, space="PSUM") as ps_pv,
            tc.tile_pool(name="ps_dn", bufs=1, space="PSUM") as ps_dn,
            tc.tile_pool(name="ps_m", bufs=1, space="PSUM") as ps_m,
        ):
            def body(_it=None):
                # ---------- tiles ----------
                xb_t = io.tile([128, CT * N], DB, tag="xb", name="xb")
                x8_t = io.tile([128, CT * N], D8, tag="x8t", name="x8t")
                xh_t = [io.tile([128, L], DT, tag=f"xh{t}", name=f"xh{t}") for t in range(CT)]
                w_in = {}
                for nm in ("q", "k", "v", "p"):
                    w_in[nm] = io.tile([128, CT * C], DB, tag=f"w{nm}", name=f"w{nm}")
                b6_t = [io.tile([128, 14], DT, tag=f"b6{t}", name=f"b6{t}") for t in range(CT)]
                gt_t = io.tile([GROUPS, CT * 128], DT, tag="gt", name="gt")

                w_s = {nm: kvq.tile([128, CT * C], DB, tag=f"ws{nm}", name=f"ws{nm}")
                       for nm in ("q", "k")}
                w8v = kvq.tile([128, CT * C], D8, tag="w8v", name="w8v")
                w8p = kvq.tile([128, CT * C], D8, tag="w8p", name="w8p")
                k8_t = kvq.tile([128, CT * N], D8, tag="k8", name="k8")
                q8_t = kvq.tile([128, CT * L], D8, tag="q8", name="q8")
                # single vt8 in PSUM-natural layout [p, jt*256 + ct*128 + c']
                vt8_t = kvq.tile([128, NJT * 256], D8, tag="vt8", name="vt8")
                vt8_r = vt8_t[:].rearrange("p (a c) -> p a c", c=128)
                # all-ones DR stationary: the denominator matmul reduces over
                # j AND broadcasts the sum to all 128 output partitions
                ones8 = wpool.tile([128, 256], D8, tag="ones8", name="ones8")
                nc.vector.memset(ones8[:], 1.0)
                shift_t = wpool.tile([128, 1], DT, tag="shift", name="shift")
                nc.vector.memset(shift_t[:], SHIFT)
                eps_t = wpool.tile([GROUPS, 1], DT, tag="eps_t", name="eps_t")
                nc.vector.memset(eps_t[:], EPS)

                # ---------- loads ----------
                # xb split across both queues (first priority), then weights,
                # then the fp32 residual halves (needed only at finales).
                nc.scalar.dma_start(b6_t[0][:], bias6_d[0])
                nc.scalar.dma_start(b6_t[1][:], bias6_d[1])
                nc.scalar.dma_start(gt_t[:], gt_d[:])
                for a in range(8):
                    q = nc.sync if a % 2 == 0 else nc.scalar
                    q.dma_start(xb_t[:, ts(a, 1024)], xb_d[:, ts(a, 1024)])
                for a in range(4):
                    q = nc.sync if a % 2 == 0 else nc.scalar
                    q.dma_start(x8_t[:, ts(a, 2048)], x8_d[:, ts(a, 2048)])
                for i, (nm, wd) in enumerate((("q", wq_d), ("k", wk_d),
                                              ("v", wv_d), ("p", wp_d))):
                    q = nc.sync if i % 2 == 0 else nc.scalar
                    q.dma_start(w_in[nm][:], wd[:])
                for a in range(4):
                    nc.sync.dma_start(xh_t[0][:, ts(a, 512)], xh_d[0, :, ts(a, 512)])
                    nc.scalar.dma_start(xh_t[1][:, ts(a, 512)], xh_d[1, :, ts(a, 512)])

                b_in = {nm: [b6_t[t][:, i:i + 1] for t in range(CT)]
                        for i, nm in enumerate(("q", "k", "v", "p", "gw", "gb"))}
                g_t = [b6_t[t][:, 6:14] for t in range(CT)]

                # HAM warmers: keep the PE activity monitor at full clock
                # through the DMA/stats window.
                for a in range(4):
                    ps_w = ps_m.tile([128, 512], DT, tag="ps_m", name="ps_w")
                    nc.tensor.matmul(
                        ps_w[:], xb_t[:, a * 2048: a * 2048 + 128],
                        xb_t[:, a * 2048: a * 2048 + 512],
                        start=True, stop=True,
                    )

                # ---------- GroupNorm stats (bn_stats, every other column) --
                parts = [wpool.tile([128, 2], DT, tag=f"parts{t}", name=f"parts{t}") for t in range(CT)]
                bns_t = [wpool.tile([128, 4 * 6], DT, tag=f"bns{t}", name=f"bns{t}") for t in range(CT)]
                for a in range(8):
                    t, la = divmod(a, 4)
                    nc.vector.bn_stats(
                        bns_t[t][:, la * 6:(la + 1) * 6],
                        xb_t[:, t * N + la * 1024: t * N + (la + 1) * 1024: 2],
                    )
                for t in range(CT):
                    mv = wpool.tile([128, 2], DT, tag="mv", name=f"mv{t}")
                    nc.vector.bn_aggr(mv[:], bns_t[t][:].rearrange("p (a s) -> p a s", s=6))
                    # parts = [mean_c, ex2_c = var_c + mean_c^2]
                    nc.vector.tensor_mul(parts[t][:, 1:2], mv[:, 0:1], mv[:, 0:1])
                    nc.vector.tensor_add(parts[t][:, 1:2], parts[t][:, 1:2], mv[:, 1:2])
                    nc.vector.tensor_copy(parts[t][:, 0:1], mv[:, 0:1])

                # group stats via indicator matmul (fp32): (8,2) = 32*[mean_g, ex2_g]
                ps_g = ps_m.tile([128, 512], DT, tag="ps_m", name="ps_g")
                for t in range(CT):
                    nc.tensor.matmul(
                        ps_g[:GROUPS, 0:2], g_t[t], parts[t][:],
                        start=(t == 0), stop=(t == CT - 1),
                    )
                st_mv = wpool.tile([GROUPS, 2], DT, tag="st_mv", name="st_mv")
                nc.vector.tensor_scalar_mul(st_mv[:], ps_g[:GROUPS, 0:2], 1.0 / 32)
                st_var = wpool.tile([GROUPS, 1], DT, tag="st_var", name="st_var")
                nc.vector.tensor_mul(st_var[:], st_mv[:, 0:1], st_mv[:, 0:1])
                nc.vector.tensor_sub(st_var[:], st_mv[:, 1:2], st_var[:])
                st2 = wpool.tile([GROUPS, 2], DT, tag="st2", name="st2")
                nc.vector.tensor_copy(st2[:, 0:1], st_mv[:, 0:1])
                st_sd = wpool.tile([GROUPS, 1], DT, tag="st_sd", name="st_sd")
                nc.scalar.activation(st_sd[:], st_var[:], AF.Sqrt, bias=eps_t[:])
                nc.vector.reciprocal(st2[:, 1:2], st_sd[:])

                # broadcast to channels (fp32 matmul): psum (128,2) = GT^T @ st2
                scale_c = [wpool.tile([128, 1], DT, tag=f"scale_c{t}", name=f"scale_c{t}") for t in range(CT)]
                bias_c = [wpool.tile([128, 1], DT, tag=f"bias_c{t}", name=f"bias_c{t}") for t in range(CT)]
                bias_cb = [wpool.tile([128, 1], DB, tag=f"bias_cb{t}", name=f"bias_cb{t}") for t in range(CT)]
                for t in range(CT):
                    ps_bc = ps_m.tile([128, 512], DT, tag="ps_m", name="ps_bc")
                    nc.tensor.matmul(ps_bc[:, 0:2], gt_t[:, ts(t, 128)], st2[:], start=True, stop=True)
                    nc.vector.tensor_mul(scale_c[t][:], b_in["gw"][t], ps_bc[:, 1:2])
                    nc.vector.tensor_mul(bias_c[t][:], ps_bc[:, 0:1], scale_c[t][:])
                    nc.vector.tensor_sub(bias_c[t][:], b_in["gb"][t], bias_c[t][:])
                    nc.vector.tensor_copy(bias_cb[t][:], bias_c[t][:])

                # ---------- fold GN scale into QKV weights ----------
                for nm in ("q", "k"):
                    for t in range(CT):
                        nc.vector.tensor_scalar_mul(
                            w_s[nm][:, ts(t, C)], w_in[nm][:, ts(t, C)], scale_c[t][:]
                        )
                for t in range(CT):
                    nc.vector.tensor_scalar_mul(
                        w8v[:, ts(t, C)], w_in["v"][:, ts(t, C)], scale_c[t][:]
                    )
                nc.vector.tensor_copy(w8p[:], w_in["p"][:])

                # ---------- bias folds (bf16 matmuls, fp32 psum) ----------
                b_f = {}
                for nm in ("q", "k", "v"):
                    b_f[nm] = []
                    for ot in range(CT):
                        ps_f = ps_m.tile([128, 512], DT, tag="ps_m", name="ps_f")
                        for ct in range(CT):
                            nc.tensor.matmul(
                                ps_f[:, 0:1],
                                w_in[nm][:, ct * C + ot * 128: ct * C + ot * 128 + 128],
                                bias_cb[ct][:],
                                start=(ct == 0), stop=(ct == CT - 1),
                            )
                        bf = wpool.tile([128, 1], DT, tag=f"bf{nm}{ot}", name=f"bf{nm}{ot}")
                        nc.vector.tensor_add(bf[:], b_in[nm][ot], ps_f[:, 0:1])
                        b_f[nm].append(bf)
                bfv_b = [wpool.tile([128, 1], DB, tag=f"bfvb{t}", name=f"bfvb{t}") for t in range(CT)]
                for t in range(CT):
                    nc.vector.tensor_copy(bfv_b[t][:], b_f["v"][t][:])
                b_f["p"] = []
                for ot in range(CT):
                    ps_f2 = ps_m.tile([128, 512], DT, tag="ps_m", name="ps_f2")
                    for ct in range(CT):
                        nc.tensor.matmul(
                            ps_f2[:, 0:1],
                            w_in["p"][:, ct * C + ot * 128: ct * C + ot * 128 + 128],
                            bfv_b[ct][:],
                            start=(ct == 0), stop=(ct == CT - 1),
                        )
                    bf = wpool.tile([128, 1], DT, tag=f"bfp{ot}", name=f"bfp{ot}")
                    nc.vector.tensor_add(bf[:], b_in["p"][ot], ps_f2[:, 0:1])
                    b_f["p"].append(bf)

                # ---------- K/Q/V, all pre-attention ----------
                # [128,512]-granular units through a 6-bank psum rotation
                # (ps_big x2 + pspv0 + pspv1 + ps_dn + ps_m are all free
                # before the first attention chunk). Early-needed evacs go
                # to ACT (they gate the first exp anyway, and ACT's queue is
                # in-order); late ones trail on DVE under the exp stream.
                x8r = x8_t[:].rearrange("p (t n) -> p t n", t=CT)
                w8vr = w8v[:].rearrange("p (t o) -> p t o", t=CT)
                qkv_slot = [0]

                def qkv_ps():
                    i = qkv_slot[0] % 6
                    qkv_slot[0] += 1
                    if i < 2:
                        return ps_big.tile([128, 1024], DT, tag="ps_big",
                                           name="ps_qkv")
                    if i < 4:
                        return ps_pv.tile([128, 512], DT, tag=f"pspv{i - 2}",
                                          name="ps_qkv")
                    if i == 4:
                        return ps_dn.tile([128, 512], DT, tag="ps_dn",
                                          name="ps_qkv")
                    return ps_m.tile([128, 512], DT, tag="ps_m", name="ps_qkv")

                def emit_kq_half(nm, dst8, ot, g, s, on_act):
                    base = ot * (N if nm == "k" else L) + g * 1024 + s * 512
                    ps = qkv_ps()
                    for ct in range(CT):
                        nc.tensor.matmul(
                            ps[:, 0:512],
                            w_s[nm][:, ct * C + ot * 128: ct * C + ot * 128 + 128],
                            xb_t[:, ct * N + g * 1024 + s * 512:
                                 ct * N + g * 1024 + (s + 1) * 512],
                            start=(ct == 0), stop=(ct == CT - 1),
                        )
                    if on_act:
                        nc.scalar.activation(dst8[:, base: base + 512], ps[:, 0:512],
                                             AF.Identity, bias=b_f[nm][ot][:])
                    else:
                        nc.vector.tensor_scalar_add(dst8[:, base: base + 512],
                                                    ps[:, 0:512], b_f[nm][ot][:])

                def emit_v_jt(jt, on_act):
                    ps = qkv_ps()
                    nc.tensor.matmul(
                        ps[:, 0:256], x8r[:, :, ts(jt, 128)], w8vr[:, :, :],
                        start=True, stop=True, perf_mode=DRM,
                    )
                    dst = vt8_t[:, jt * 256:(jt + 1) * 256]
                    if on_act:
                        nc.scalar.copy(dst, ps[:, 0:256])
                    else:
                        nc.vector.tensor_copy(dst, ps[:, 0:256])

                for ot in range(CT):            # K g0, Q g0 -> ACT
                    for s in range(2):
                        emit_kq_half("k", k8_t, ot, 0, s, True)
                for ot in range(CT):
                    for s in range(2):
                        emit_kq_half("q", q8_t, ot, 0, s, True)
                for jt in range(8):             # V j-tiles 0-7 -> ACT
                    emit_v_jt(jt, True)
                for g in range(1, 4):           # K g1-3 + V 8-31 -> DVE
                    for ot in range(CT):
                        for s in range(2):
                            emit_kq_half("k", k8_t, ot, g, s, False)
                    for jt in range(8 * g, 8 * (g + 1)):
                        emit_v_jt(jt, False)
                for ot in range(CT):            # Q g1 -> DVE
                    for s in range(2):
                        emit_kq_half("q", q8_t, ot, 1, s, False)

                k8r = k8_t[:].rearrange("p (t n) -> p t n", t=CT)
                q8r = q8_t[:].rearrange("p (t n) -> p t n", t=CT)

                if debug:
                    dmp = io.tile([128, 1024], DT, tag="dmp", name="dmp")
                    dsc = io.tile([128, 4], DT, tag="dsc", name="dsc")
                    for t in range(CT):
                        nc.vector.tensor_copy(dsc[:, t:t + 1], scale_c[t][:])
                        nc.vector.tensor_copy(dsc[:, 2 + t:3 + t], bias_c[t][:])
                    nc.sync.dma_start(dbg_d["d_scale"][:], dsc[:])

                # ---------- attention over i-chunks ----------
                for ic in range(NCH):
                    ps_pv_t = [ps_pv.tile([128, IC], DT, tag=f"pspv{ct}", name=f"pspv{ct}")
                               for ct in range(CT)]
                    ps_den = ps_dn.tile([128, IC], DT, tag="ps_dn", name="ps_den")

                    def emit_s(jp):
                        ps_sc = ps_big.tile([128, 1024], DT, tag="ps_big", name="ps_sc")
                        for q in range(2):
                            nc.tensor.matmul(
                                ps_sc[:, ts(q, 512)],
                                k8r[:, :, ts(2 * jp + q, 128)],
                                q8r[:, :, ic * IC: (ic + 1) * IC],
                                start=True, stop=True, perf_mode=DRM,
                            )
                        pt = ptp.tile([128, 1024], D8, tag="pt", name="pt")
                        nc.scalar.activation(pt[:], ps_sc[:], AF.Exp, scale=SCALE,
                                             bias=shift_t[:])
                        return pt

                    def emit_pv(jp, pt):
                        ptr = pt[:].rearrange("p (q i) -> p q i", q=2)
                        for ct in range(CT):
                            nc.tensor.matmul(
                                ps_pv_t[ct][:],
                                vt8_r[:, 4 * jp + ct: 4 * jp + ct + 3: 2, :],
                                ptr[:, :, :],
                                start=(jp == 0), stop=(jp == NJP - 1),
                                perf_mode=DRM,
                            )
                        nc.tensor.matmul(
                            ps_den[:],
                            ones8[:].rearrange("p (q m) -> p q m", q=2),
                            ptr[:, :, :],
                            start=(jp == 0), stop=(jp == NJP - 1),
                            perf_mode=DRM,
                        )

                    pt_prev = emit_s(0)
                    for jp in range(1, NJP):
                        pt_cur = emit_s(jp)
                        emit_pv(jp - 1, pt_prev)
                        pt_prev = pt_cur
                    emit_pv(NJP - 1, pt_prev)

                    # denominator is already partition-broadcast; reciprocal it
                    rb_sb = mis.tile([128, IC], DT, tag="rb_sb", name="rb_sb")
                    nc.vector.reciprocal(rb_sb[:], ps_den[:])

                    ao8 = mis.tile([128, 2 * IC], D8, tag="ao8", name="ao8")
                    for ct in range(CT):
                        nc.vector.tensor_mul(ao8[:, ts(ct, IC)], ps_pv_t[ct][:], rb_sb[:])

                    if debug and ic == 0:
                        nc.vector.tensor_copy(dmp[:, 0:IC], ps_den[:])
                        nc.sync.dma_start(dbg_d["d_den"][:], dmp[:, 0:IC])
                        nc.vector.tensor_copy(dmp[:], ao8[:])
                        nc.sync.dma_start(dbg_d["d_ao"][:], dmp[:])
                        nc.vector.tensor_copy(dmp[:], pt_prev[:])
                        nc.sync.dma_start(dbg_d["d_pt"][:], dmp[:])

                    ao8r = ao8[:].rearrange("p (t i) -> p t i", t=CT)
                    w8pr = w8p[:].rearrange("p (t o) -> p t o", t=CT)
                    for ot in range(CT):
                        ps_y = ps_m.tile([128, 512], DT, tag="ps_m", name="ps_y")
                        nc.tensor.matmul(
                            ps_y[:], w8pr[:, :, ts(ot, 128)],
                            ao8r[:, :, :],
                            start=True, stop=True, perf_mode=DRM,
                        )
                        y_sb = mis.tile([128, IC], DT, tag="y_sb", name="y_sb")
                        nc.vector.scalar_tensor_tensor(
                            y_sb[:], ps_y[:], b_f["p"][ot][:],
                            xh_t[ot][:, ts(ic, IC)],
                            op0=OP.add, op1=OP.add,
                        )
                        q = nc.sync if ot == 0 else nc.scalar
                        q.dma_start(y_d[ot, :, ts(ic, IC)], y_sb[:])

                if debug:
                    for a in range(8):
                        nc.vector.tensor_copy(dmp[:], k8_t[:, ts(a, 1024)])
                        nc.sync.dma_start(dbg_d["d_k8"][:, ts(a, 1024)], dmp[:])
                    for a in range(4):
                        nc.vector.tensor_copy(dmp[:], q8_t[:, ts(a, 1024)])
                        nc.sync.dma_start(dbg_d["d_q8"][:, ts(a, 1024)], dmp[:])
                    nc.vector.tensor_copy(dmp[:], vt8_t[:, 0:1024])
                    nc.sync.dma_start(dbg_d["d_vt8"][:], dmp[:])

            if repeat == 1:
                body()
            else:
                hints = (mybir.EngineType.PE, mybir.EngineType.Activation,
                         mybir.EngineType.DVE, mybir.EngineType.SP)
                with tc.For_i(0, repeat, 1, hint_engines=hints) as it:
                    body(it)

    if split:
        split_waits(nc)
    return nc


# ---------------- host-side sharding helpers ----------------

def make_in_maps(inputs):
    fb = mybir.dt.np(DB)

    x = np.asarray(inputs["x"], dtype=np.float32)
    n = x.shape[0]

    def wt(name):
        w = np.asarray(inputs[name], dtype=np.float32)
        # [p, t*C + o] = w[o, t*128+p]
        return np.ascontiguousarray(
            w.T.reshape(CT, 128, C).transpose(1, 0, 2).reshape(128, CT * C)
        ).astype(fb)

    wqT, wkT, wvT, wpT = wt("wq"), wt("wk"), wt("wv"), wt("wp")

    bias6 = np.zeros((CT, 128, 14), dtype=np.float32)
    for i, nm in enumerate(("bq", "bk", "bv", "bp", "gn_w", "gn_b")):
        bias6[:, :, i] = np.asarray(inputs[nm], np.float32).reshape(CT, 128)
    for t in range(CT):
        for p in range(128):
            bias6[t, p, 6 + (t * 128 + p) // 32] = 1.0  # G indicator
    GT = np.zeros((GROUPS, CT * 128), dtype=np.float32)
    for c in range(C):
        GT[c // 32, c] = 1.0

    in_maps = []
    xb_cache = {}
    for core in range(2 * n):
        b, h = divmod(core, 2)
        xb = x[b].reshape(CT, 128, N)
        key = (b, h)
        if key not in xb_cache:
            # pre-rolled so the program's query columns [0, L) are this
            # half's queries; keys are permutation-invariant
            xr = np.roll(xb, -h * L, axis=2) if h else xb
            flat = np.ascontiguousarray(
                xr.transpose(1, 0, 2).reshape(128, CT * N))
            xb_cache[key] = (flat.astype(fb), flat.astype(mybir.dt.np(D8)))
        xh = np.ascontiguousarray(xb[:, :, h * L:(h + 1) * L])
        in_maps.append({
            "xb": xb_cache[key][0],
            "x8": xb_cache[key][1],
            "xh": xh,
            "wqT": wqT, "wkT": wkT, "wvT": wvT, "wpT": wpT,
            "bias6": bias6, "GT": GT,
        })
    return in_maps


def assemble(results, n=4):
    out = np.zeros((n, C, 64, 64), dtype=np.float32)
    flat = out.reshape(n, C, N)
    for core, res in enumerate(results):
        b, h = divmod(core, 2)
        flat[b, :, h * L:(h + 1) * L] = res["y"].reshape(C, L)
    return out


_CACHE = {}


def kernel(**inputs) -> np.ndarray:
    n = np.asarray(inputs["x"]).shape[0]
    n_cores = 2 * n
    if "nc" not in _CACHE:
        _CACHE["nc"] = build(split=True, repeat=1)
    nc = _CACHE["nc"]
    in_maps = make_in_maps(inputs)
    last_err = None
    for _attempt in range(2):  # one retry on transient axon/RPC failures
        try:
            res = run_bass_kernel_spmd(nc, in_maps, list(range(n_cores)))
            return assemble(res.results, n=n)
        except Exception as e:  # noqa: BLE001
            last_err = e
    raise last_err



# revision 10
# speedup vs baseline: 1.0457x; 1.0457x over previous
"""AttentionBlock (GroupNorm + single-head self-attention + residual) as a
Bass/Tile kernel for one Trainium2 chip (8 NeuronCores), SPMD data-parallel.

Redesign of the fp8 baseline around the ACT-engine exp stream (the hard
floor: 64 x [128,1024] Exp instructions ~= 60us of ACT time):

- V projection is eliminated: out = Wp.(V.P/den) with V = Wv.Xn collapses to
  W_eff.(Xn.P)/den where W_eff = Wp.Wv.diag(s) (Wp@Wv precomputed on host,
  GN scale s folded on-chip). Xn.P = diag(s).(x.P) + bias_c (x) den, and the
  bias terms fold into the output bias:
      out += Wpv.bias_c + Wp.bv + bp   (bp + Wp@bv precomputed on host).
  x.P uses a host-pre-transposed fp8 x (x8T, [j, c] layout) as the matmul
  stationary, so the entire V production + evacuation pipeline of the
  baseline (32 matmuls + 32 PSUM evacs + w8v fold) disappears.
- The exp stream starts ~10us in (vs ~50us): DMA is batched (xb in 4 big
  transfers first, weights fused into one transfer), GroupNorm stats run on
  sampled columns as chunks land, K g0-g1 + Q(chunk 0) are prefetched
  through a 4-bank PSUM rotation, and the remaining K/Q production is
  interleaved into chunk 0/1's jp loop via a dedicated production bank.
- Scores are pre-issued 2 jp ahead (across chunk boundaries too) so ACT
  never waits at a chunk seam; the finale (reciprocal/ao8/out-proj/residual
  add) runs entirely off ACT and overlaps the next chunk's exps.
- ACT executes ONLY the stats sqrt + the 64 exps (one table switch);
  every PSUM evacuation is on DVE.
- PSUM: scores 2x[128,1024] (4 banks) + XP 2x[128,512] + den [128,512]
  (reused by out-proj at finales) + 1 production bank = 16KB/partition.

Sharding: 4 images x 2 query-halves -> 8 cores. x is pre-rolled per half
on the host (keys are permutation-invariant); residual/output use the
original column range h*L..(h+1)*L.
"""

import numpy as np

import bass_rust
import concourse.bass as bass
import concourse.mybir as mybir
import concourse.tile as tile
from concourse.bass import ts
from concourse.bass_utils import run_bass_kernel_spmd

# ---------------------------------------------------------------------------
# walrus single-sync-wait workaround (same as baseline)

_counter = [0]


def _mk_nop(engine, wait):
    _counter[0] += 1
    nop = mybir.InstNoOp(name=f"WSPLIT-{_counter[0]}", ins=[], outs=[])
    nop.engine = engine
    nop.sync_info = bass_rust.SyncInfo(on_wait=[wait], on_update=[])
    return nop


def split_waits(nc, verbose=False):
    f = nc.m.functions[0]
    new_blocks = []
    n_split = 0
    for blk in f.blocks:
        insts = blk.instructions
        out = []
        for inst in insts:
            si = inst.sync_info
            if si is not None and si.on_wait and len(si.on_wait) > 1:
                waits = list(si.on_wait)
                for w in waits[1:]:
                    out.append(_mk_nop(inst.engine, w))
                si.on_wait = waits[:1]
                n_split += 1
            out.append(inst)
        new_blocks.append(bass_rust.BasicBlock(name=blk.name, instructions=out))
    f.blocks = new_blocks
    if verbose:
        print(f"split_waits: split {n_split} instructions")
    return n_split


# ---------------------------------------------------------------------------

DT = mybir.dt.float32
DB = mybir.dt.bfloat16
D8 = mybir.dt.float8e4
AF = mybir.ActivationFunctionType
OP = mybir.AluOpType
DRM = mybir.MatmulPerfMode.DoubleRow

C = 256
N = 4096
L = 2048
IC = 512          # i-chunk size
NCH = L // IC     # 4 chunks
NJT = N // 128    # 32 j-tiles
NJP = NJT // 2    # 16 j-tile pairs
CT = C // 128     # 2 channel tiles
GROUPS = 8
EPS = 1e-5
SCALE = C ** -0.5
SHIFT = -4.5


def build(split=True, repeat=1, prec=None, debug=False):
    nc = bass.Bass()
    dbg_d = {}
    if debug:
        dbg_d["d_k8"] = nc.declare_dram_parameter("d_k8", [128, CT * N], DT, isOutput=True)
        dbg_d["d_q8"] = nc.declare_dram_parameter("d_q8", [128, CT * L], DT, isOutput=True)
        dbg_d["d_pt"] = nc.declare_dram_parameter("d_pt", [128, 1024], DT, isOutput=True)
        dbg_d["d_den"] = nc.declare_dram_parameter("d_den", [128, IC], DT, isOutput=True)
        dbg_d["d_ao"] = nc.declare_dram_parameter("d_ao", [128, 2 * IC], DT, isOutput=True)

    # xb: full image, bf16, ct-major free dim: [p, t*N + j] = x[t*128+p, j]
    xb_d = nc.declare_dram_parameter("xb", [128, CT * N], DB, isOutput=False)
    # x8T: transposed fp8 x: [p, jt*256 + t*128 + c] = x[t*128+c, jt*128+p]
    x8t_d = nc.declare_dram_parameter("x8T", [128, NJT * C], D8, isOutput=False)
    # xh: fp32 residual slice (this core's query half): [t, p, i]
    xh_d = nc.declare_dram_parameter("xh", [CT, 128, L], DT, isOutput=False)
    # weights wq|wk|wpv fused, transposed block layout:
    #   [p, w*(CT*C) + t*C + o] = W[o, t*128+p]
    w3_d = nc.declare_dram_parameter("w3", [128, 3 * CT * C], DB, isOutput=False)
    # packed small params, t-major: [p, t*13 + i], i = bq bk bpc gnw gnb G(8)
    bias6_d = nc.declare_dram_parameter("bias6", [128, CT * 13], DT, isOutput=False)
    gt_d = nc.declare_dram_parameter("GT", [GROUPS, CT * 128], DT, isOutput=False)
    y_d = nc.declare_dram_parameter("y", [CT, 128, L], DT, isOutput=True)

    with tile.TileContext(nc) as tc:
        with (
            tc.tile_pool(name="io", bufs=1) as io,
            tc.tile_pool(name="wp_", bufs=1) as wpool,
            tc.tile_pool(name="kvq", bufs=1) as kvq,
            tc.tile_pool(name="ptp", bufs=8) as ptp,
            tc.tile_pool(name="mis", bufs=4) as mis,
            tc.tile_pool(name="ps_big", bufs=2, space="PSUM") as ps_big,
            tc.tile_pool(name="ps_xp", bufs=1, space="PSUM") as ps_xp,
            tc.tile_pool(name="ps_dn", bufs=1, space="PSUM") as ps_dn,
            tc.tile_pool(name="ps_k", bufs=1, space="PSUM") as ps_k,
        ):
            def body(_it=None):
                # ---------- tiles ----------
                xb_t = io.tile([128, CT * N], DB, tag="xb", name="xb")
                x8t_t = io.tile([128, NJT * C], D8, tag="x8t", name="x8t")
                x8t_r = x8t_t[:].rearrange("p (a c) -> p a c", c=128)
                xh_t = [io.tile([128, L], DT, tag=f"xh{t}", name=f"xh{t}") for t in range(CT)]
                w3_t = io.tile([128, 3 * CT * C], DB, tag="w3", name="w3")
                w_in = {nm: w3_t[:, i * CT * C:(i + 1) * CT * C]
                        for i, nm in enumerate(("q", "k", "pv"))}
                b6_t = io.tile([128, CT * 13], DT, tag="b6", name="b6")
                gt_t = io.tile([GROUPS, CT * 128], DT, tag="gt", name="gt")

                w_s = {nm: kvq.tile([128, CT * C], DB, tag=f"ws{nm}", name=f"ws{nm}")
                       for nm in ("q", "k")}
                w8pv = kvq.tile([128, CT * C], D8, tag="w8pv", name="w8pv")
                k8_t = kvq.tile([128, CT * N], D8, tag="k8", name="k8")
                q8_t = kvq.tile([128, CT * L], D8, tag="q8", name="q8")
                # all-ones DR stationary: the denominator matmul reduces over
                # j AND broadcasts the sum to all 128 output partitions
                ones8 = wpool.tile([128, 256], D8, tag="ones8", name="ones8")
                nc.vector.memset(ones8[:], 1.0)
                shift_t = wpool.tile([128, 1], DT, tag="shift", name="shift")
                nc.vector.memset(shift_t[:], SHIFT)
                eps_t = wpool.tile([GROUPS, 1], DT, tag="eps_t", name="eps_t")
                nc.vector.memset(eps_t[:], EPS)

                # ---------- loads ----------
                # xb first in 4 big chunks on both queues; then the small
                # params + fused weights; x8T and the residual trail.
                for a in range(4):
                    q = nc.sync if a % 2 == 0 else nc.scalar
                    q.dma_start(xb_t[:, ts(a, 2048)], xb_d[:, ts(a, 2048)])
                nc.sync.dma_start(b6_t[:], bias6_d[:])
                nc.scalar.dma_start(gt_t[:], gt_d[:])
                nc.sync.dma_start(w3_t[:], w3_d[:])
                for a in range(2):
                    q = nc.sync if a % 2 == 0 else nc.scalar
                    q.dma_start(x8t_t[:, ts(a, 4096)], x8t_d[:, ts(a, 4096)])
                nc.sync.dma_start(xh_t[0][:], xh_d[0])
                nc.scalar.dma_start(xh_t[1][:], xh_d[1])

                b_in = {nm: [b6_t[:, t * 13 + i: t * 13 + i + 1] for t in range(CT)]
                        for i, nm in enumerate(("q", "k", "pc", "gw", "gb"))}
                g_t = [b6_t[:, t * 13 + 5: (t + 1) * 13] for t in range(CT)]

                # HAM warmers: keep the PE activity monitor at full clock
                # through the DMA/stats window.
                for a in range(4):
                    ps_w = ps_k.tile([128, 512], DT, tag="ps_k", name="ps_w")
                    nc.tensor.matmul(
                        ps_w[:], xb_t[:, a * 2048: a * 2048 + 128],
                        xb_t[:, a * 2048: a * 2048 + 512],
                        start=True, stop=True,
                    )

                # ---------- GroupNorm stats (bn_stats, every other column) --
                parts = [wpool.tile([128, 2], DT, tag=f"parts{t}", name=f"parts{t}") for t in range(CT)]
                bns_t = [wpool.tile([128, 4 * 6], DT, tag=f"bns{t}", name=f"bns{t}") for t in range(CT)]
                for a in range(8):
                    t, la = divmod(a, 4)
                    nc.vector.bn_stats(
                        bns_t[t][:, la * 6:(la + 1) * 6],
                        xb_t[:, t * N + la * 1024: t * N + (la + 1) * 1024: 2],
                    )
                for t in range(CT):
                    mv = wpool.tile([128, 2], DT, tag="mv", name=f"mv{t}")
                    nc.vector.bn_aggr(mv[:], bns_t[t][:].rearrange("p (a s) -> p a s", s=6))
                    # parts = [mean_c, ex2_c = var_c + mean_c^2]
                    nc.vector.tensor_mul(parts[t][:, 1:2], mv[:, 0:1], mv[:, 0:1])
                    nc.vector.tensor_add(parts[t][:, 1:2], parts[t][:, 1:2], mv[:, 1:2])
                    nc.vector.tensor_copy(parts[t][:, 0:1], mv[:, 0:1])

                # group stats via indicator matmul (fp32): (8,2) = 32*[mean_g, ex2_g]
                ps_g = ps_k.tile([128, 512], DT, tag="ps_k", name="ps_g")
                for t in range(CT):
                    nc.tensor.matmul(
                        ps_g[:GROUPS, 0:2], g_t[t], parts[t][:],
                        start=(t == 0), stop=(t == CT - 1),
                    )
                st_mv = wpool.tile([GROUPS, 2], DT, tag="st_mv", name="st_mv")
                nc.vector.tensor_scalar_mul(st_mv[:], ps_g[:GROUPS, 0:2], 1.0 / 32)
                st_var = wpool.tile([GROUPS, 1], DT, tag="st_var", name="st_var")
                nc.vector.tensor_mul(st_var[:], st_mv[:, 0:1], st_mv[:, 0:1])
                nc.vector.tensor_sub(st_var[:], st_mv[:, 1:2], st_var[:])
                st2 = wpool.tile([GROUPS, 2], DT, tag="st2", name="st2")
                nc.vector.tensor_copy(st2[:, 0:1], st_mv[:, 0:1])
                st_sd = wpool.tile([GROUPS, 1], DT, tag="st_sd", name="st_sd")
                nc.scalar.activation(st_sd[:], st_var[:], AF.Sqrt, bias=eps_t[:])
                nc.vector.reciprocal(st2[:, 1:2], st_sd[:])

                # broadcast to channels (fp32 matmul): psum (128,2) = GT^T @ st2
                scale_c = [wpool.tile([128, 1], DT, tag=f"scale_c{t}", name=f"scale_c{t}") for t in range(CT)]
                bias_c = [wpool.tile([128, 1], DT, tag=f"bias_c{t}", name=f"bias_c{t}") for t in range(CT)]
                bias_cb = [wpool.tile([128, 1], DB, tag=f"bias_cb{t}", name=f"bias_cb{t}") for t in range(CT)]
                for t in range(CT):
                    ps_bc = ps_k.tile([128, 512], DT, tag="ps_k", name="ps_bc")
                    nc.tensor.matmul(ps_bc[:, 0:2], gt_t[:, ts(t, 128)], st2[:], start=True, stop=True)
                    nc.vector.tensor_mul(scale_c[t][:], b_in["gw"][t], ps_bc[:, 1:2])
                    nc.vector.tensor_mul(bias_c[t][:], ps_bc[:, 0:1], scale_c[t][:])
                    nc.vector.tensor_sub(bias_c[t][:], b_in["gb"][t], bias_c[t][:])
                    nc.vector.tensor_copy(bias_cb[t][:], bias_c[t][:])

                # ---------- fold GN scale into K/Q/PV weights ----------
                for nm in ("q", "k"):
                    for t in range(CT):
                        nc.vector.tensor_scalar_mul(
                            w_s[nm][:, ts(t, C)], w_in[nm][:, ts(t, C)], scale_c[t][:]
                        )
                for t in range(CT):
                    nc.vector.tensor_scalar_mul(
                        w8pv[:, ts(t, C)], w_in["pv"][:, ts(t, C)], scale_c[t][:]
                    )

                # ---------- bias folds (bf16 matmuls, fp32 psum) ----------
                # b_f[q/k] = b + W^T @ gn_bias ; b_f[p] = bpc + Wpv^T @ gn_bias
                b_f = {}
                for nm, wsrc, badd in (("q", w_in["q"], "q"), ("k", w_in["k"], "k"),
                                       ("p", w_in["pv"], "pc")):
                    b_f[nm] = []
                    for ot in range(CT):
                        ps_f = ps_k.tile([128, 512], DT, tag="ps_k", name="ps_f")
                        for ct in range(CT):
                            nc.tensor.matmul(
                                ps_f[:, 0:1],
                                wsrc[:, ct * C + ot * 128: ct * C + ot * 128 + 128],
                                bias_cb[ct][:],
                                start=(ct == 0), stop=(ct == CT - 1),
                            )
                        bf = wpool.tile([128, 1], DT, tag=f"bf{nm}{ot}", name=f"bf{nm}{ot}")
                        nc.vector.tensor_add(bf[:], b_in[badd][ot], ps_f[:, 0:1])
                        b_f[nm].append(bf)

                # ---------- K/Q production units ----------
                # one unit = 512 cols of K or Q for one output channel block,
                # through a caller-chosen PSUM bank; evacuation on DVE.
                def emit_kq_half(nm, dst8, ot, g, s, pstag):
                    base = ot * (N if nm == "k" else L) + g * 1024 + s * 512
                    if pstag == "xp0" or pstag == "xp1":
                        ps = ps_xp.tile([128, 512], DT, tag=pstag, name="ps_kq")
                    elif pstag == "dn":
                        ps = ps_dn.tile([128, 512], DT, tag="ps_dn", name="ps_kq")
                    else:
                        ps = ps_k.tile([128, 512], DT, tag="ps_k", name="ps_kq")
                    for ct in range(CT):
                        nc.tensor.matmul(
                            ps[:, 0:512],
                            w_s[nm][:, ct * C + ot * 128: ct * C + ot * 128 + 128],
                            xb_t[:, ct * N + g * 1024 + s * 512:
                                 ct * N + g * 1024 + (s + 1) * 512],
                            start=(ct == 0), stop=(ct == CT - 1),
                        )
                    nc.vector.tensor_scalar_add(dst8[:, base: base + 512],
                                                ps[:, 0:512], b_f[nm][ot][:])

                # in-loop production for chunk 0 (K g2-g3, then Q s1 for
                # chunk 1); chunk 1 produces Q g1 (chunks 2-3).
                loop_units = {0: [], 1: [], 2: [], 3: []}
                for (nm, g, s) in (("k", 2, 0), ("k", 2, 1), ("k", 3, 0), ("k", 3, 1), ("q", 0, 1)):
                    for ot in range(CT):
                        loop_units[0].append((nm, g, s, ot))
                for (nm, g, s) in (("q", 1, 0), ("q", 1, 1)):
                    for ot in range(CT):
                        loop_units[1].append((nm, g, s, ot))

                k8r = k8_t[:].rearrange("p (t n) -> p t n", t=CT)
                q8r = q8_t[:].rearrange("p (t n) -> p t n", t=CT)

                # ---------- attention over i-chunks ----------
                def emit_s(ic, jp):
                    ps_sc = ps_big.tile([128, 1024], DT, tag="ps_big", name="ps_sc")
                    for q in range(2):
                        nc.tensor.matmul(
                            ps_sc[:, ts(q, 512)],
                            k8r[:, :, ts(2 * jp + q, 128)],
                            q8r[:, :, ic * IC: (ic + 1) * IC],
                            start=True, stop=True, perf_mode=DRM,
                        )
                    pt = ptp.tile([128, 1024], D8, tag="pt", name="pt")
                    nc.scalar.activation(pt[:], ps_sc[:], AF.Exp, scale=SCALE,
                                         bias=shift_t[:])
                    return pt

                dbg_tiles = {}
                if debug:
                    dbg_tiles["dmp"] = io.tile([128, 1024], DT, tag="dmp", name="dmp")

                # prefetch: K g0+g1 and Q s0 (chunk 0's queries). The first
                # four units gate exp(0), so scores(0,0/1) are issued right
                # behind them; the last units avoid the xp banks so chunk-0's
                # XP/den accumulators aren't WAW-blocked on their evacs.
                pre_units = []
                for (nm, g, s) in (("k", 0, 0), ("q", 0, 0), ("k", 0, 1), ("k", 1, 0), ("k", 1, 1)):
                    for ot in range(CT):
                        pre_units.append((nm, g, s, ot))
                pre_tags = ["xp0", "xp1", "dn", "k", "xp0", "xp1", "dn", "k", "dn", "k"]
                for i in range(4):
                    nm, g, s, ot = pre_units[i]
                    emit_kq_half(nm, k8_t if nm == "k" else q8_t, ot, g, s, pre_tags[i])
                pts0 = [emit_s(0, 0), emit_s(0, 1)]
                for i in range(4, len(pre_units)):
                    nm, g, s, ot = pre_units[i]
                    emit_kq_half(nm, k8_t if nm == "k" else q8_t, ot, g, s, pre_tags[i])

                for ic in range(NCH):
                    ps_xp_t = [ps_xp.tile([128, IC], DT, tag=f"xp{ct}", name=f"psxp{ct}")
                               for ct in range(CT)]
                    ps_den = ps_dn.tile([128, IC], DT, tag="ps_dn", name="ps_den")

                    def emit_xp(jp, pt):
                        ptr = pt[:].rearrange("p (q i) -> p q i", q=2)
                        for ct in range(CT):
                            nc.tensor.matmul(
                                ps_xp_t[ct][:],
                                x8t_r[:, 4 * jp + ct: 4 * jp + ct + 3: 2, :],
                                ptr[:, :, :],
                                start=(jp == 0), stop=(jp == NJP - 1),
                                perf_mode=DRM,
                            )
                        nc.tensor.matmul(
                            ps_den[:],
                            ones8[:].rearrange("p (q m) -> p q m", q=2),
                            ptr[:, :, :],
                            start=(jp == 0), stop=(jp == NJP - 1),
                            perf_mode=DRM,
                        )

                    if ic == 0:
                        pts = pts0
                    else:
                        pts = pts_next  # noqa: F821  (set by previous chunk)

                    units = loop_units[ic]
                    ui = 0
                    for jp in range(NJP):
                        # pre-issue scores 2 ahead (crossing into next chunk)
                        if jp < NJP - 2:
                            pts.append(emit_s(ic, jp + 2))
                        elif ic < NCH - 1:
                            if jp == NJP - 2:
                                pts_next = [emit_s(ic + 1, 0)]
                            else:
                                pts_next.append(emit_s(ic + 1, 1))
                        if ui < len(units):
                            nm, g, s, ot = units[ui]
                            ui += 1
                            dst = k8_t if nm == "k" else q8_t
                            emit_kq_half(nm, dst, ot, g, s, "k")
                        emit_xp(jp, pts[jp])
                    pt_last = pts[NJP - 1]

                    # ---------- finale (no ACT involvement) ----------
                    rb_sb = mis.tile([128, IC], DT, tag="rb_sb", name="rb_sb")
                    nc.vector.reciprocal(rb_sb[:], ps_den[:])
                    ao8 = mis.tile([128, 2 * IC], D8, tag="ao8", name="ao8")
                    for ct in range(CT):
                        nc.vector.tensor_mul(ao8[:, ts(ct, IC)], ps_xp_t[ct][:], rb_sb[:])

                    if debug and ic == 0:
                        dmp = dbg_tiles["dmp"]
                        nc.vector.tensor_copy(dmp[:, 0:IC], ps_den[:])
                        nc.sync.dma_start(dbg_d["d_den"][:], dmp[:, 0:IC])
                        nc.vector.tensor_copy(dmp[:], ao8[:])
                        nc.sync.dma_start(dbg_d["d_ao"][:], dmp[:])
                        nc.vector.tensor_copy(dmp[:], pt_last[:])
                        nc.sync.dma_start(dbg_d["d_pt"][:], dmp[:])

                    ao8r = ao8[:].rearrange("p (t i) -> p t i", t=CT)
                    w8pvr = w8pv[:].rearrange("p (t o) -> p t o", t=CT)
                    for ot in range(CT):
                        # out-proj reuses the den bank (free after reciprocal)
                        ps_y = ps_dn.tile([128, IC], DT, tag="ps_dn", name="ps_y")
                        nc.tensor.matmul(
                            ps_y[:], w8pvr[:, :, ts(ot, 128)],
                            ao8r[:, :, :],
                            start=True, stop=True, perf_mode=DRM,
                        )
                        y_sb = mis.tile([128, IC], DT, tag="y_sb", name="y_sb")
                        nc.vector.scalar_tensor_tensor(
                            y_sb[:], ps_y[:], b_f["p"][ot][:],
                            xh_t[ot][:, ts(ic, IC)],
                            op0=OP.add, op1=OP.add,
                        )
                        q = nc.sync if ot == 0 else nc.scalar
                        q.dma_start(y_d[ot, :, ts(ic, IC)], y_sb[:])

                if debug:
                    dmp = dbg_tiles["dmp"]
                    for a in range(8):
                        nc.vector.tensor_copy(dmp[:], k8_t[:, ts(a, 1024)])
                        nc.sync.dma_start(dbg_d["d_k8"][:, ts(a, 1024)], dmp[:])
                    for a in range(4):
                        nc.vector.tensor_copy(dmp[:], q8_t[:, ts(a, 1024)])
                        nc.sync.dma_start(dbg_d["d_q8"][:, ts(a, 1024)], dmp[:])

            if repeat == 1:
                body()
            else:
                hints = (mybir.EngineType.PE, mybir.EngineType.Activation,
                         mybir.EngineType.DVE, mybir.EngineType.SP)
                with tc.For_i(0, repeat, 1, hint_engines=hints) as it:
                    body(it)

    if split:
        split_waits(nc)
    return nc


# ---------------- host-side sharding helpers ----------------

def make_in_maps(inputs):
    fb = mybir.dt.np(DB)
    f8 = mybir.dt.np(D8)

    x = np.asarray(inputs["x"], dtype=np.float32)
    n = x.shape[0]

    def wt(w):
        # [p, t*C + o] = w[o, t*128+p]
        return np.ascontiguousarray(
            w.T.reshape(CT, 128, C).transpose(1, 0, 2).reshape(128, CT * C)
        )

    wq = np.asarray(inputs["wq"], np.float32)
    wk = np.asarray(inputs["wk"], np.float32)
    wv = np.asarray(inputs["wv"], np.float32)
    wp = np.asarray(inputs["wp"], np.float32)
    wpv = wp @ wv
    w3 = np.concatenate([wt(wq), wt(wk), wt(wpv)], axis=1).astype(fb)

    bpc = (np.asarray(inputs["bp"], np.float32)
           + wp @ np.asarray(inputs["bv"], np.float32))
    bias6 = np.zeros((128, CT * 13), dtype=np.float32)
    for i, v in enumerate((inputs["bq"], inputs["bk"], bpc,
                           inputs["gn_w"], inputs["gn_b"])):
        vv = np.asarray(v, np.float32).reshape(CT, 128)
        for t in range(CT):
            bias6[:, t * 13 + i] = vv[t]
    for t in range(CT):
        for p in range(128):
            bias6[p, t * 13 + 5 + (t * 128 + p) // 32] = 1.0  # G indicator
    GT = np.zeros((GROUPS, CT * 128), dtype=np.float32)
    for c in range(C):
        GT[c // 32, c] = 1.0

    in_maps = []
    xb_cache = {}
    for core in range(2 * n):
        b, h = divmod(core, 2)
        xb = x[b].reshape(CT, 128, N)
        key = (b, h)
        if key not in xb_cache:
            # pre-rolled so the program's query columns [0, L) are this
            # half's queries; keys are permutation-invariant
            xr = np.roll(xb, -h * L, axis=2) if h else xb
            flat = np.ascontiguousarray(
                xr.transpose(1, 0, 2).reshape(128, CT * N))
            # x8T[p, jt*256 + t*128 + c] = xr[t, c, jt*128+p]
            xt = xr.reshape(C, N).T  # [j, c] (c = t*128 + cc)
            x8t = np.ascontiguousarray(
                xt.reshape(NJT, 128, C).transpose(1, 0, 2).reshape(128, NJT * C))
            xb_cache[key] = (flat.astype(fb), x8t.astype(f8))
        xh = np.ascontiguousarray(xb[:, :, h * L:(h + 1) * L])
        in_maps.append({
            "xb": xb_cache[key][0],
            "x8T": xb_cache[key][1],
            "xh": xh,
            "w3": w3,
            "bias6": bias6, "GT": GT,
        })
    return in_maps


def assemble(results, n=4):
    out = np.zeros((n, C, 64, 64), dtype=np.float32)
    flat = out.reshape(n, C, N)
    for core, res in enumerate(results):
        b, h = divmod(core, 2)
        flat[b, :, h * L:(h + 1) * L] = res["y"].reshape(C, L)
    return out


_CACHE = {}


def kernel(**inputs) -> np.ndarray:
    n = np.asarray(inputs["x"]).shape[0]
    n_cores = 2 * n
    if "nc" not in _CACHE:
        _CACHE["nc"] = build(split=True, repeat=1)
    nc = _CACHE["nc"]
    in_maps = make_in_maps(inputs)
    last_err = None
    for _attempt in range(2):  # one retry on transient axon/RPC failures
        try:
            res = run_bass_kernel_spmd(nc, in_maps, list(range(n_cores)))
            return assemble(res.results, n=n)
        except Exception as e:  # noqa: BLE001
            last_err = e
    raise last_err


# revision 13
# speedup vs baseline: 1.2173x; 1.1641x over previous
"""AttentionBlock (GroupNorm + single-head self-attention + residual) as a
Bass/Tile kernel for one Trainium2 chip (8 NeuronCores), SPMD data-parallel.

v3 — PE-throughput-oriented revision. HW microbenchmarks show this part's
real rates: PE matmul ~= 60ns + 0.574ns/moving-col (no DoublePixel), ACT
exp ~= 292ns + 0.87ns/col, DVE psum-evac ~= 1.86ns/col. That makes PE the
bottleneck (scores + XP + den ~= 113us of moving columns), so v3 minimizes
PE column work and the serial ramp:

- V projection eliminated (out = W_eff.(x.P)/den, W_eff = Wp.Wv.diag(s),
  x.P uses host-pre-transposed fp8 x8T as stationary; biases fold exactly).
- K/Q projections run in fp8 DoubleRow (contraction 256 in one pass):
  halves production column count vs bf16. Stats, K/Q, and XP all read the
  same fp8 data, which the numpy error model puts at ~6.5e-3 rel err.
- The bf16 x copy is gone entirely: GroupNorm stats come from the fp8
  c-major x8 (every 4th column), so the stats chain clears ~1.5us after
  x8's 1MB DMA (first transfer in flight). DMA total is 4.9MB.
- Ramp: the 4 production units gating exp(0) evacuate in parallel on
  DVE + GpSimd; scores are pre-issued 2 jp ahead (across chunk seams).
- Remaining K/Q production interleaves into chunk 0/1's jp loop through a
  dedicated PSUM bank; all steady-state evacuations on DVE; ACT does only
  the stats sqrt + 64 exps.
- Finales run off ACT; the last chunk's two out-proj matmuls use the
  (by then free) score banks to shorten the tail.

Sharding: 4 images x 2 query-halves -> 8 cores. x is pre-rolled per half
on the host (keys are permutation-invariant); residual/output use the
original column range h*L..(h+1)*L.
"""

import numpy as np

import bass_rust
import concourse.bass as bass
import concourse.mybir as mybir
import concourse.tile as tile
from concourse.bass import ts
from concourse.bass_utils import run_bass_kernel_spmd

# ---------------------------------------------------------------------------
# walrus single-sync-wait workaround (same as baseline)

_counter = [0]


def _mk_nop(engine, wait):
    _counter[0] += 1
    nop = mybir.InstNoOp(name=f"WSPLIT-{_counter[0]}", ins=[], outs=[])
    nop.engine = engine
    nop.sync_info = bass_rust.SyncInfo(on_wait=[wait], on_update=[])
    return nop


def split_waits(nc, verbose=False):
    f = nc.m.functions[0]
    new_blocks = []
    n_split = 0
    for blk in f.blocks:
        insts = blk.instructions
        out = []
        for inst in insts:
            si = inst.sync_info
            if si is not None and si.on_wait and len(si.on_wait) > 1:
                waits = list(si.on_wait)
                for w in waits[1:]:
                    out.append(_mk_nop(inst.engine, w))
                si.on_wait = waits[:1]
                n_split += 1
            out.append(inst)
        new_blocks.append(bass_rust.BasicBlock(name=blk.name, instructions=out))
    f.blocks = new_blocks
    if verbose:
        print(f"split_waits: split {n_split} instructions")
    return n_split


# ---------------------------------------------------------------------------

DT = mybir.dt.float32
DB = mybir.dt.bfloat16
D8 = mybir.dt.float8e4
AF = mybir.ActivationFunctionType
OP = mybir.AluOpType
DRM = mybir.MatmulPerfMode.DoubleRow

C = 256
N = 4096
L = 2048
IC = 512          # i-chunk size
NCH = L // IC     # 4 chunks
NJT = N // 128    # 32 j-tiles
NJP = NJT // 2    # 16 j-tile pairs
CT = C // 128     # 2 channel tiles
GROUPS = 8
EPS = 1e-5
SCALE = C ** -0.5
SHIFT = -4.5


def build(split=True, repeat=1, prec=None, debug=False):
    nc = bass.Bass()
    dbg_d = {}
    if debug:
        dbg_d["d_k8"] = nc.declare_dram_parameter("d_k8", [128, CT * N], DT, isOutput=True)
        dbg_d["d_q8"] = nc.declare_dram_parameter("d_q8", [128, CT * L], DT, isOutput=True)
        dbg_d["d_pt"] = nc.declare_dram_parameter("d_pt", [128, 1024], DT, isOutput=True)
        dbg_d["d_den"] = nc.declare_dram_parameter("d_den", [128, IC], DT, isOutput=True)
        dbg_d["d_ao"] = nc.declare_dram_parameter("d_ao", [128, 2 * IC], DT, isOutput=True)

    # x8: fp8 image, ct-major free dim: [p, t*N + j] = x[t*128+p, j]
    x8_d = nc.declare_dram_parameter("x8", [128, CT * N], D8, isOutput=False)
    # x8T: transposed fp8 x: [p, jt*256 + t*128 + c] = x[t*128+c, jt*128+p]
    x8t_d = nc.declare_dram_parameter("x8T", [128, NJT * C], D8, isOutput=False)
    # xh: fp32 residual slice (this core's query half): [t, p, i]
    xh_d = nc.declare_dram_parameter("xh", [CT, 128, L], DT, isOutput=False)
    # weights wq|wk|wpv fused, transposed block layout:
    #   [p, w*(CT*C) + t*C + o] = W[o, t*128+p]
    w3_d = nc.declare_dram_parameter("w3", [128, 3 * CT * C], DB, isOutput=False)
    # packed small params, t-major: [p, t*13 + i], i = bq bk bpc gnw gnb G(8)
    bias6_d = nc.declare_dram_parameter("bias6", [128, CT * 13], DT, isOutput=False)
    gt_d = nc.declare_dram_parameter("GT", [GROUPS, CT * 128], DT, isOutput=False)
    y_d = nc.declare_dram_parameter("y", [CT, 128, L], DT, isOutput=True)

    with tile.TileContext(nc) as tc:
        with (
            tc.tile_pool(name="io", bufs=1) as io,
            tc.tile_pool(name="wp_", bufs=1) as wpool,
            tc.tile_pool(name="kvq", bufs=1) as kvq,
            tc.tile_pool(name="ptp", bufs=8) as ptp,
            tc.tile_pool(name="mis", bufs=4) as mis,
            tc.tile_pool(name="ps_big", bufs=2, space="PSUM") as ps_big,
            tc.tile_pool(name="ps_xp", bufs=1, space="PSUM") as ps_xp,
            tc.tile_pool(name="ps_dn", bufs=1, space="PSUM") as ps_dn,
            tc.tile_pool(name="ps_k", bufs=1, space="PSUM") as ps_k,
        ):
            def body(_it=None):
                # ---------- tiles ----------
                x8_t = io.tile([128, CT * N], D8, tag="x8", name="x8")
                x8r = x8_t[:].rearrange("p (t n) -> p t n", t=CT)
                x8t_t = io.tile([128, NJT * C], D8, tag="x8t", name="x8t")
                x8t_r = x8t_t[:].rearrange("p (a c) -> p a c", c=128)
                xh_t = [io.tile([128, L], DT, tag=f"xh{t}", name=f"xh{t}") for t in range(CT)]
                w3_t = io.tile([128, 3 * CT * C], DB, tag="w3", name="w3")
                w_in = {nm: w3_t[:, i * CT * C:(i + 1) * CT * C]
                        for i, nm in enumerate(("q", "k", "pv"))}
                b6_t = io.tile([128, CT * 13], DT, tag="b6", name="b6")
                gt_t = io.tile([GROUPS, CT * 128], DT, tag="gt", name="gt")

                w8 = {nm: kvq.tile([128, CT * C], D8, tag=f"w8{nm}", name=f"w8{nm}")
                      for nm in ("q", "k", "pv")}
                w8r = {nm: w8[nm][:].rearrange("p (t o) -> p t o", t=CT)
                       for nm in ("q", "k", "pv")}
                k8_t = kvq.tile([128, CT * N], D8, tag="k8", name="k8")
                q8_t = kvq.tile([128, CT * L], D8, tag="q8", name="q8")
                # all-ones DR stationary: the denominator matmul reduces over
                # j AND broadcasts the sum to all 128 output partitions
                ones8 = wpool.tile([128, 256], D8, tag="ones8", name="ones8")
                nc.vector.memset(ones8[:], 1.0)
                shift_t = wpool.tile([128, 1], DT, tag="shift", name="shift")
                nc.vector.memset(shift_t[:], SHIFT)
                eps_t = wpool.tile([GROUPS, 1], DT, tag="eps_t", name="eps_t")
                nc.vector.memset(eps_t[:], EPS)

                # ---------- loads ----------
                # x8 first (stats + K/Q production + warmers), then small
                # params + wq/wk, then x8T, wpv, and the residual.
                for a in range(4):
                    q = nc.sync if a % 2 == 0 else nc.scalar
                    q.dma_start(x8_t[:, ts(a, 2048)], x8_d[:, ts(a, 2048)])
                nc.sync.dma_start(b6_t[:], bias6_d[:])
                nc.scalar.dma_start(gt_t[:], gt_d[:])
                nc.sync.dma_start(w3_t[:, 0:2 * CT * C], w3_d[:, 0:2 * CT * C])
                for a in range(2):
                    q = nc.scalar if a % 2 == 0 else nc.sync
                    q.dma_start(x8t_t[:, ts(a, 4096)], x8t_d[:, ts(a, 4096)])
                nc.scalar.dma_start(w3_t[:, 2 * CT * C:], w3_d[:, 2 * CT * C:])
                nc.sync.dma_start(xh_t[0][:], xh_d[0])
                nc.scalar.dma_start(xh_t[1][:], xh_d[1])

                b_in = {nm: [b6_t[:, t * 13 + i: t * 13 + i + 1] for t in range(CT)]
                        for i, nm in enumerate(("q", "k", "pc", "gw", "gb"))}
                g_t = [b6_t[:, t * 13 + 5: (t + 1) * 13] for t in range(CT)]

                # HAM warmers: keep the PE activity monitor at full clock
                # through the DMA/stats window.
                for a in range(4):
                    ps_w = ps_k.tile([128, 512], DT, tag="ps_k", name="ps_w")
                    nc.tensor.matmul(
                        ps_w[:], x8r[:, :, a * 1024: a * 1024 + 128],
                        x8r[:, :, a * 1024: a * 1024 + 512],
                        start=True, stop=True, perf_mode=DRM,
                    )

                # ---------- GroupNorm stats (bn_stats on fp8, every 4th col) -
                parts = [wpool.tile([128, 2], DT, tag=f"parts{t}", name=f"parts{t}") for t in range(CT)]
                bns_t = [wpool.tile([128, 4 * 6], DT, tag=f"bns{t}", name=f"bns{t}") for t in range(CT)]
                for a in range(8):
                    t, la = divmod(a, 4)
                    nc.vector.bn_stats(
                        bns_t[t][:, la * 6:(la + 1) * 6],
                        x8_t[:, t * N + la * 1024: t * N + (la + 1) * 1024: 4],
                    )
                for t in range(CT):
                    mv = wpool.tile([128, 2], DT, tag="mv", name=f"mv{t}")
                    nc.vector.bn_aggr(mv[:], bns_t[t][:].rearrange("p (a s) -> p a s", s=6))
                    # parts = [mean_c, ex2_c = var_c + mean_c^2]
                    nc.vector.tensor_mul(parts[t][:, 1:2], mv[:, 0:1], mv[:, 0:1])
                    nc.vector.tensor_add(parts[t][:, 1:2], parts[t][:, 1:2], mv[:, 1:2])
                    nc.vector.tensor_copy(parts[t][:, 0:1], mv[:, 0:1])

                # group stats via indicator matmul (fp32): (8,2) = 32*[mean_g, ex2_g]
                ps_g = ps_k.tile([128, 512], DT, tag="ps_k", name="ps_g")
                for t in range(CT):
                    nc.tensor.matmul(
                        ps_g[:GROUPS, 0:2], g_t[t], parts[t][:],
                        start=(t == 0), stop=(t == CT - 1),
                    )
                st_mv = wpool.tile([GROUPS, 2], DT, tag="st_mv", name="st_mv")
                nc.vector.tensor_scalar_mul(st_mv[:], ps_g[:GROUPS, 0:2], 1.0 / 32)
                st_var = wpool.tile([GROUPS, 1], DT, tag="st_var", name="st_var")
                nc.vector.tensor_mul(st_var[:], st_mv[:, 0:1], st_mv[:, 0:1])
                nc.vector.tensor_sub(st_var[:], st_mv[:, 1:2], st_var[:])
                st2 = wpool.tile([GROUPS, 2], DT, tag="st2", name="st2")
                nc.vector.tensor_copy(st2[:, 0:1], st_mv[:, 0:1])
                st_sd = wpool.tile([GROUPS, 1], DT, tag="st_sd", name="st_sd")
                nc.scalar.activation(st_sd[:], st_var[:], AF.Sqrt, bias=eps_t[:])
                nc.vector.reciprocal(st2[:, 1:2], st_sd[:])

                # broadcast to channels (fp32 matmul): psum (128,2) = GT^T @ st2
                scale_c = [wpool.tile([128, 1], DT, tag=f"scale_c{t}", name=f"scale_c{t}") for t in range(CT)]
                bias_c = [wpool.tile([128, 1], DT, tag=f"bias_c{t}", name=f"bias_c{t}") for t in range(CT)]
                bias_cb = [wpool.tile([128, 1], DB, tag=f"bias_cb{t}", name=f"bias_cb{t}") for t in range(CT)]
                for t in range(CT):
                    ps_bc = ps_k.tile([128, 512], DT, tag="ps_k", name="ps_bc")
                    nc.tensor.matmul(ps_bc[:, 0:2], gt_t[:, ts(t, 128)], st2[:], start=True, stop=True)
                    nc.vector.tensor_mul(scale_c[t][:], b_in["gw"][t], ps_bc[:, 1:2])
                    nc.vector.tensor_mul(bias_c[t][:], ps_bc[:, 0:1], scale_c[t][:])
                    nc.vector.tensor_sub(bias_c[t][:], b_in["gb"][t], bias_c[t][:])
                    nc.vector.tensor_copy(bias_cb[t][:], bias_c[t][:])

                # ---------- fold GN scale into K/Q/PV weights (fp8 out) -----
                for nm in ("q", "k", "pv"):
                    for t in range(CT):
                        nc.vector.tensor_scalar_mul(
                            w8[nm][:, ts(t, C)], w_in[nm][:, ts(t, C)], scale_c[t][:]
                        )

                # ---------- bias folds (bf16 matmuls, fp32 psum) ----------
                # b_f[q/k] = b + W^T @ gn_bias ; b_f[p] = bpc + Wpv^T @ gn_bias
                b_f = {}
                for nm, wsrc, badd in (("q", w_in["q"], "q"), ("k", w_in["k"], "k"),
                                       ("p", w_in["pv"], "pc")):
                    b_f[nm] = []
                    for ot in range(CT):
                        ps_f = ps_k.tile([128, 512], DT, tag="ps_k", name="ps_f")
                        for ct in range(CT):
                            nc.tensor.matmul(
                                ps_f[:, 0:1],
                                wsrc[:, ct * C + ot * 128: ct * C + ot * 128 + 128],
                                bias_cb[ct][:],
                                start=(ct == 0), stop=(ct == CT - 1),
                            )
                        bf = wpool.tile([128, 1], DT, tag=f"bf{nm}{ot}", name=f"bf{nm}{ot}")
                        nc.vector.tensor_add(bf[:], b_in[badd][ot], ps_f[:, 0:1])
                        b_f[nm].append(bf)

                # ---------- K/Q production units (fp8 DoubleRow) ----------
                # one unit = 512 cols of K or Q for one output channel block;
                # single DR matmul, evacuation on DVE (or GpSimd in the ramp).
                def emit_kq_half(nm, dst8, ot, g, s, pstag, evac=None):
                    base = ot * (N if nm == "k" else L) + g * 1024 + s * 512
                    if pstag == "xp0" or pstag == "xp1":
                        ps = ps_xp.tile([128, 512], DT, tag=pstag, name="ps_kq")
                    elif pstag == "dn":
                        ps = ps_dn.tile([128, 512], DT, tag="ps_dn", name="ps_kq")
                    else:
                        ps = ps_k.tile([128, 512], DT, tag="ps_k", name="ps_kq")
                    nc.tensor.matmul(
                        ps[:, 0:512],
                        w8r[nm][:, :, ot * 128:(ot + 1) * 128],
                        x8r[:, :, g * 1024 + s * 512: g * 1024 + (s + 1) * 512],
                        start=True, stop=True, perf_mode=DRM,
                    )
                    bf = b_f["q" if nm == "q" else nm][ot][:]
                    if evac is nc.scalar:
                        nc.scalar.activation(dst8[:, base: base + 512],
                                             ps[:, 0:512], AF.Identity, bias=bf)
                    else:
                        nc.vector.tensor_scalar_add(dst8[:, base: base + 512],
                                                    ps[:, 0:512], bf)

                # in-loop production for chunk 0 (K g2-g3, then Q s1 for
                # chunk 1); chunk 1 produces Q g1 (chunks 2-3).
                loop_units = {0: [], 1: [], 2: [], 3: []}
                for (nm, g, s) in (("k", 2, 0), ("k", 2, 1), ("k", 3, 0), ("k", 3, 1), ("q", 0, 1)):
                    for ot in range(CT):
                        loop_units[0].append((nm, g, s, ot))
                for (nm, g, s) in (("q", 1, 0), ("q", 1, 1)):
                    for ot in range(CT):
                        loop_units[1].append((nm, g, s, ot))

                k8r = k8_t[:].rearrange("p (t n) -> p t n", t=CT)
                q8r = q8_t[:].rearrange("p (t n) -> p t n", t=CT)

                # ---------- attention over i-chunks ----------
                def emit_s(ic, jp):
                    ps_sc = ps_big.tile([128, 1024], DT, tag="ps_big", name="ps_sc")
                    for q in range(2):
                        nc.tensor.matmul(
                            ps_sc[:, ts(q, 512)],
                            k8r[:, :, ts(2 * jp + q, 128)],
                            q8r[:, :, ic * IC: (ic + 1) * IC],
                            start=True, stop=True, perf_mode=DRM,
                        )
                    pt = ptp.tile([128, 1024], D8, tag="pt", name="pt")
                    nc.scalar.activation(pt[:], ps_sc[:], AF.Exp, scale=SCALE,
                                         bias=shift_t[:])
                    return pt

                dbg_tiles = {}
                if debug:
                    dbg_tiles["dmp"] = io.tile([128, 1024], DT, tag="dmp", name="dmp")

                # prefetch: K g0+g1 and Q s0 (chunk 0's queries). The first
                # four units gate exp(0) and evacuate on DVE+GpSimd in
                # parallel; scores(0,0/1) issue right behind them. The last
                # units avoid the xp banks so chunk-0's XP/den accumulators
                # aren't WAW-blocked on their evacs.
                pre_units = []
                for (nm, g, s) in (("k", 0, 0), ("q", 0, 0), ("k", 0, 1), ("k", 1, 0), ("k", 1, 1)):
                    for ot in range(CT):
                        pre_units.append((nm, g, s, ot))
                pre_tags = ["xp0", "xp1", "dn", "k", "xp0", "xp1", "dn", "k", "dn", "k"]
                pre_evac = [nc.vector, nc.scalar, nc.vector, nc.scalar,
                            nc.vector, nc.vector, nc.vector, nc.vector,
                            nc.vector, nc.vector]
                for i in range(4):
                    nm, g, s, ot = pre_units[i]
                    emit_kq_half(nm, k8_t if nm == "k" else q8_t, ot, g, s,
                                 pre_tags[i], pre_evac[i])
                pts0 = [emit_s(0, 0), emit_s(0, 1)]
                for i in range(4, len(pre_units)):
                    nm, g, s, ot = pre_units[i]
                    emit_kq_half(nm, k8_t if nm == "k" else q8_t, ot, g, s,
                                 pre_tags[i], pre_evac[i])

                for ic in range(NCH):
                    ps_xp_t = [ps_xp.tile([128, IC], DT, tag=f"xp{ct}", name=f"psxp{ct}")
                               for ct in range(CT)]
                    ps_den = ps_dn.tile([128, IC], DT, tag="ps_dn", name="ps_den")

                    def emit_xp(jp, pt):
                        ptr = pt[:].rearrange("p (q i) -> p q i", q=2)
                        for ct in range(CT):
                            nc.tensor.matmul(
                                ps_xp_t[ct][:],
                                x8t_r[:, 4 * jp + ct: 4 * jp + ct + 3: 2, :],
                                ptr[:, :, :],
                                start=(jp == 0), stop=(jp == NJP - 1),
                                perf_mode=DRM,
                            )
                        nc.tensor.matmul(
                            ps_den[:],
                            ones8[:].rearrange("p (q m) -> p q m", q=2),
                            ptr[:, :, :],
                            start=(jp == 0), stop=(jp == NJP - 1),
                            perf_mode=DRM,
                        )

                    if ic == 0:
                        pts = pts0
                    else:
                        pts = pts_next  # noqa: F821  (set by previous chunk)

                    units = loop_units[ic]
                    ui = 0
                    for jp in range(NJP):
                        # pre-issue scores 2 ahead (crossing into next chunk)
                        if jp < NJP - 2:
                            pts.append(emit_s(ic, jp + 2))
                        elif ic < NCH - 1:
                            if jp == NJP - 2:
                                pts_next = [emit_s(ic + 1, 0)]
                            else:
                                pts_next.append(emit_s(ic + 1, 1))
                        if ui < len(units):
                            nm, g, s, ot = units[ui]
                            ui += 1
                            dst = k8_t if nm == "k" else q8_t
                            emit_kq_half(nm, dst, ot, g, s, "k")
                        emit_xp(jp, pts[jp])
                    pt_last = pts[NJP - 1]

                    # ---------- finale (no ACT involvement) ----------
                    rb_sb = mis.tile([128, IC], DT, tag="rb_sb", name="rb_sb")
                    nc.vector.reciprocal(rb_sb[:], ps_den[:])
                    ao8 = mis.tile([128, 2 * IC], D8, tag="ao8", name="ao8")
                    for ct in range(CT):
                        nc.vector.tensor_mul(ao8[:, ts(ct, IC)], ps_xp_t[ct][:], rb_sb[:])

                    if debug and ic == 0:
                        dmp = dbg_tiles["dmp"]
                        nc.vector.tensor_copy(dmp[:, 0:IC], ps_den[:])
                        nc.sync.dma_start(dbg_d["d_den"][:], dmp[:, 0:IC])
                        nc.vector.tensor_copy(dmp[:], ao8[:])
                        nc.sync.dma_start(dbg_d["d_ao"][:], dmp[:])
                        nc.vector.tensor_copy(dmp[:], pt_last[:])
                        nc.sync.dma_start(dbg_d["d_pt"][:], dmp[:])

                    ao8r = ao8[:].rearrange("p (t i) -> p t i", t=CT)
                    for ot in range(CT):
                        if ic == NCH - 1:
                            # score banks are free now: run both out-proj
                            # matmuls in parallel to shorten the tail
                            ps_y = ps_big.tile([128, 1024], DT, tag="ps_big",
                                               name="ps_y")[:, 0:IC]
                        else:
                            # out-proj reuses the den bank (free after recip)
                            ps_y = ps_dn.tile([128, IC], DT, tag="ps_dn",
                                              name="ps_y")[:]
                        nc.tensor.matmul(
                            ps_y, w8r["pv"][:, :, ts(ot, 128)],
                            ao8r[:, :, :],
                            start=True, stop=True, perf_mode=DRM,
                        )
                        y_sb = mis.tile([128, IC], DT, tag="y_sb", name="y_sb")
                        nc.vector.scalar_tensor_tensor(
                            y_sb[:], ps_y, b_f["p"][ot][:],
                            xh_t[ot][:, ts(ic, IC)],
                            op0=OP.add, op1=OP.add,
                        )
                        q = nc.sync if ot == 0 else nc.scalar
                        q.dma_start(y_d[ot, :, ts(ic, IC)], y_sb[:])

                if debug:
                    dmp = dbg_tiles["dmp"]
                    for a in range(8):
                        nc.vector.tensor_copy(dmp[:], k8_t[:, ts(a, 1024)])
                        nc.sync.dma_start(dbg_d["d_k8"][:, ts(a, 1024)], dmp[:])
                    for a in range(4):
                        nc.vector.tensor_copy(dmp[:], q8_t[:, ts(a, 1024)])
                        nc.sync.dma_start(dbg_d["d_q8"][:, ts(a, 1024)], dmp[:])

            if repeat == 1:
                body()
            else:
                hints = (mybir.EngineType.PE, mybir.EngineType.Activation,
                         mybir.EngineType.DVE, mybir.EngineType.SP)
                with tc.For_i(0, repeat, 1, hint_engines=hints) as it:
                    body(it)

    if split:
        split_waits(nc)
    return nc


# ---------------- host-side sharding helpers ----------------

def make_in_maps(inputs):
    fb = mybir.dt.np(DB)
    f8 = mybir.dt.np(D8)

    x = np.asarray(inputs["x"], dtype=np.float32)
    n = x.shape[0]

    def wt(w):
        # [p, t*C + o] = w[o, t*128+p]
        return np.ascontiguousarray(
            w.T.reshape(CT, 128, C).transpose(1, 0, 2).reshape(128, CT * C)
        )

    wq = np.asarray(inputs["wq"], np.float32)
    wk = np.asarray(inputs["wk"], np.float32)
    wv = np.asarray(inputs["wv"], np.float32)
    wp = np.asarray(inputs["wp"], np.float32)
    wpv = wp @ wv
    w3 = np.concatenate([wt(wq), wt(wk), wt(wpv)], axis=1).astype(fb)

    bpc = (np.asarray(inputs["bp"], np.float32)
           + wp @ np.asarray(inputs["bv"], np.float32))
    bias6 = np.zeros((128, CT * 13), dtype=np.float32)
    for i, v in enumerate((inputs["bq"], inputs["bk"], bpc,
                           inputs["gn_w"], inputs["gn_b"])):
        vv = np.asarray(v, np.float32).reshape(CT, 128)
        for t in range(CT):
            bias6[:, t * 13 + i] = vv[t]
    for t in range(CT):
        for p in range(128):
            bias6[p, t * 13 + 5 + (t * 128 + p) // 32] = 1.0  # G indicator
    GT = np.zeros((GROUPS, CT * 128), dtype=np.float32)
    for c in range(C):
        GT[c // 32, c] = 1.0

    in_maps = []
    xb_cache = {}
    for core in range(2 * n):
        b, h = divmod(core, 2)
        xb = x[b].reshape(CT, 128, N)
        key = (b, h)
        if key not in xb_cache:
            # pre-rolled so the program's query columns [0, L) are this
            # half's queries; keys are permutation-invariant
            xr = np.roll(xb, -h * L, axis=2) if h else xb
            flat = np.ascontiguousarray(
                xr.transpose(1, 0, 2).reshape(128, CT * N))
            # x8T[p, jt*256 + t*128 + c] = xr[t, c, jt*128+p]
            xt = xr.reshape(C, N).T  # [j, c] (c = t*128 + cc)
            x8t = np.ascontiguousarray(
                xt.reshape(NJT, 128, C).transpose(1, 0, 2).reshape(128, NJT * C))
            xb_cache[key] = (flat.astype(f8), x8t.astype(f8))
        xh = np.ascontiguousarray(xb[:, :, h * L:(h + 1) * L])
        in_maps.append({
            "x8": xb_cache[key][0],
            "x8T": xb_cache[key][1],
            "xh": xh,
            "w3": w3,
            "bias6": bias6, "GT": GT,
        })
    return in_maps


def assemble(results, n=4):
    out = np.zeros((n, C, 64, 64), dtype=np.float32)
    flat = out.reshape(n, C, N)
    for core, res in enumerate(results):
        b, h = divmod(core, 2)
        flat[b, :, h * L:(h + 1) * L] = res["y"].reshape(C, L)
    return out


_CACHE = {}


def kernel(**inputs) -> np.ndarray:
    n = np.asarray(inputs["x"]).shape[0]
    n_cores = 2 * n
    if "nc" not in _CACHE:
        _CACHE["nc"] = build(split=True, repeat=1)
    nc = _CACHE["nc"]
    in_maps = make_in_maps(inputs)
    last_err = None
    for _attempt in range(2):  # one retry on transient axon/RPC failures
        try:
            res = run_bass_kernel_spmd(nc, in_maps, list(range(n_cores)))
            return assemble(res.results, n=n)
        except Exception as e:  # noqa: BLE001
            last_err = e
    raise last_err
